# revision 33
# baseline (speedup 1.0000x reference)
"""Trainium2 Bass kernel for nn_Attention_63745904608049 (v4).

Relative-position attention (TransformerXL-style shift, Enformer-style pos
basis), batch 1, seq 2048, d_model 1536, 8 heads x 64. Head-parallel over 8
NeuronCores; the positional-score relative shift is realized as a DRAM
roundtrip (straight write, sheared flat-AP read).

v4 (on the v2 structure):
 - host-precomputed pos projection (no pos AllGather).
 - softmax diet: no accum/normalize per tile; row sums via ones-column in V
   (M=65); per-group normalize with a partition-parallel reciprocal
   (row->col K=1 matmuls, [128,m] reciprocal, col->row identity matmul).
 - score add via identity-matmul PSUM accumulation (PE, not DVE).
 - q-tile processing order [15, 0..14]: chunk 4's AllGather fires early and
   absorbs the CC-stream setup; AV groups lag their tiles by 2 so the
   attention transposes are never on the PE critical path; fins lag AGs.
 - queue assignment: gpsimd touches only collectives + AG-gated loads;
   xt loads split across two queues; transposes alternate sync/scalar.
"""
import contextlib
import ctypes
import math
import os
import sys
import types

import numpy as np
import ml_dtypes

import concourse.bass as bass
import concourse.mybir as mybir
from concourse.tile import TileContext
from concourse.masks import make_identity
from concourse.bass_utils import run_bass_kernel_spmd

# ----------------------------------------------------------------------------
# problem constants
N = 2048
DM = 1536
H = 8
HD = 64
INNER = H * HD            # 512
NCORES = 8
QT = N // 128             # 16 query tiles
WIN = 2175                # per-q-tile pos table window (128 + 2048 - 1)
TSTRIDE = 2304            # padded row stride of the T scratch (elements)
CHUNKS = [(0, 512), (512, 512), (1024, 512), (1536, 512), (2048, 127)]
GROUPS = [(0, 4), (4, 8), (8, 12), (12, 16)]  # q-tile AV groups
SUPERS = [(0, 2), (2, 4)]  # AV-group ranges per AllGather super-chunk
F32 = mybir.dt.float32
BF16 = mybir.dt.bfloat16
FP16 = mybir.dt.float16

_LAST_RESULT = None       # BassKernelResults of the last run (for test.py)


# ----------------------------------------------------------------------------
# axon NTFF profiling hook (lets BASS_TRACE=1 produce exec_time_ns under axon)
def _install_ntff_hook(so_path="/opt/axon/libaxon_pjrt.so"):
    try:
        import antenv.axon_hooks  # noqa: F401
        return
    except ImportError:
        pass
    try:
        lib = ctypes.CDLL(so_path)
    except OSError:
        return
    if not hasattr(lib, "axon_start_nrt_profile"):
        return
    lib.axon_start_nrt_profile.argtypes = [ctypes.POINTER(ctypes.c_int64), ctypes.c_size_t]
    lib.axon_start_nrt_profile.restype = ctypes.c_int64
    lib.axon_stop_nrt_profile.argtypes = [ctypes.c_char_p]
    lib.axon_stop_nrt_profile.restype = ctypes.c_int64

    @contextlib.contextmanager
    def _hook(output_dir, device_ids):
        import jax
        jax.devices()
        if device_ids:
            ids = (ctypes.c_int64 * len(device_ids))(*device_ids)
            rc = lib.axon_start_nrt_profile(ids, len(device_ids))
        else:
            rc = lib.axon_start_nrt_profile(None, 0)
        if rc != 0:
            raise RuntimeError(f"axon_start_nrt_profile rc={rc}")
        try:
            yield
        finally:
            n = lib.axon_stop_nrt_profile(str(output_dir).encode())
            print(f"ntff profile: {n} file(s) written to {output_dir}")

    mod = types.ModuleType("antenv.axon_hooks")
    mod.get_axon_ntff_profile_hook = lambda: _hook
    mod.set_axon_ntff_profile_hook = lambda h: None
    sys.modules["antenv.axon_hooks"] = mod


_install_ntff_hook()


# ----------------------------------------------------------------------------
# BIR post-processing: this container's walrus build rejects instructions with
# more than one sync wait; split extra waits onto preceding NoOps.
def _split_waits(bir_bytes, maxw=1):
    import json
    d = json.loads(bir_bytes)
    counter = [0]
    for fn in d["functions"]:
        for blk in fn["blocks"]:
            out = []
            for ins in blk["instructions"]:
                si = ins.get("sync_info")
                waits = (si or {}).get("on_wait") or []
                if len(waits) > maxw:
                    excess = waits[:-maxw]
                    ins["sync_info"]["on_wait"] = waits[-maxw:]
                    for i in range(0, len(excess), maxw):
                        counter[0] += 1
                        nop = {
                            "engine": ins["engine"],
                            "ins": [],
                            "outs": [],
                            "name": f"I-waitsplit-{counter[0]}",
                            "opcode": "NoOp",
                            "sync_info": {"on_update": [],
                                          "on_wait": excess[i:i + maxw]},
                        }
                        if "debug" in ins:
                            nop["debug"] = ins["debug"]
                        out.append(nop)
                out.append(ins)
            blk["instructions"] = out
    return json.dumps(d).encode()


# ----------------------------------------------------------------------------
# host-side positional embedding table (pure function of N, DM)
_POS_CACHE = {}


def _pos_embed():
    if "emb" in _POS_CACHE:
        return _POS_CACHE["emb"]
    n, fs = N, DM
    nb = fs // 6  # 256
    dist = np.arange(-n + 1, n, dtype=np.float64)
    adist = np.abs(dist)[:, None]

    max_range = math.log(n) / math.log(2.0)
    half_life = 2.0 ** np.linspace(3.0, max_range, nb)
    exp_feat = np.exp(-math.log(2.0) / half_life[None, :] * adist)

    with np.errstate(over="ignore"):
        center_widths = 2.0 ** np.arange(1, nb + 1, dtype=np.float64) - 1.0
    cmask_feat = (center_widths[None, :] > adist).astype(np.float64)

    stddev = n / (2.0 * nb)
    start_mean = n / nb
    mean = np.linspace(start_mean, float(n), nb)[None, :]
    conc = (mean / stddev) ** 2
    rate = mean / stddev ** 2
    with np.errstate(divide="ignore", invalid="ignore"):
        log_unnorm = (conc - 1.0) * np.log(adist) - rate * adist
    lgamma = np.vectorize(math.lgamma)
    log_norm = lgamma(conc) - conc * np.log(rate)
    with np.errstate(invalid="ignore"):
        prob = np.exp(log_unnorm - log_norm) + 1e-08
    prob = np.nan_to_num(prob, nan=1e-08)  # adist=0: 0*inf -> use limit 0, then +eps
    # recompute the adist == 0 row exactly: log_unnorm = -inf -> exp -> 0
    zrow = np.where(adist[:, 0] == 0)[0]
    prob[zrow, :] = 1e-08
    gamma_feat = prob / prob.max(axis=-1, keepdims=True)

    emb = np.concatenate([exp_feat, cmask_feat, gamma_feat], axis=-1)
    emb = np.concatenate([emb, np.sign(dist)[:, None] * emb], axis=-1)
    emb = emb.astype(np.float32)  # (4095, 1536)
    _POS_CACHE["emb"] = emb
    return emb


# ----------------------------------------------------------------------------
# device graph (identical for all cores; per-core data differs)
_GRAPH_CACHE = {}


def _build_graph():
    if "nc" in _GRAPH_CACHE:
        return _GRAPH_CACHE["nc"]
    nc = bass.Bass()

    xT = nc.declare_dram_parameter("xT", [DM, N], FP16, isOutput=False)
    wq = nc.declare_dram_parameter("wq", [DM, HD], FP16, isOutput=False)
    wkv = nc.declare_dram_parameter("wkv", [DM, 2 * HD], FP16, isOutput=False)
    ptp = nc.declare_dram_parameter("ptp", [HD, 4096], FP16, isOutput=False)
    cbias = nc.declare_dram_parameter("cbias", [HD, 1], F32, isOutput=False)
    pbias = nc.declare_dram_parameter("pbias", [HD, 1], F32, isOutput=False)
    wout = nc.declare_dram_parameter("wout", [INNER, 192], BF16, isOutput=False)
    bout = nc.declare_dram_parameter("bout", [1, 192], F32, isOutput=False)
    out_ext = nc.declare_dram_parameter("out", [N, 192], FP16, isOutput=True)

    # internal DRAM
    t_dram = [nc.dram_tensor(f"tscratch{a}", [128, TSTRIDE], FP16) for a in range(QT)]
    oag_in = [nc.dram_tensor(f"oag_in{g}", [HD, 512], BF16)
              for g in range(len(GROUPS))]
    oag_out = [nc.dram_tensor(f"oag_out{g}", [NCORES * HD, 512], BF16,
                              addr_space="Shared") for g in range(len(GROUPS))]

    rgroups = [list(range(NCORES))]
    Act = mybir.ActivationFunctionType

    with TileContext(nc) as tc:
        with contextlib.ExitStack() as ctx:
            persist = ctx.enter_context(tc.tile_pool(name="persist", bufs=1))
            work = ctx.enter_context(tc.tile_pool(name="work", bufs=2))
            psum = ctx.enter_context(tc.tile_pool(name="psum", bufs=3, space="PSUM"))

            # ---------------- persistent tiles ----------------
            qcT = persist.tile([HD, N], FP16, tag="qcT")
            qpT = persist.tile([HD, N], FP16, tag="qpT")
            kvT = persist.tile([128, N], FP16, tag="kvT")   # k rows 0:64, v rows 64:128
            PT = persist.tile([HD, 4096], FP16, tag="PT")
            vsb = [persist.tile([128, HD + 1], BF16, tag=f"v{k}", name=f"v{k}")
                   for k in range(QT)]
            ident = persist.tile([128, 128], FP16, tag="ident")
            make_identity(nc, ident)
            identf = persist.tile([128, 128], F32, tag="identf")
            make_identity(nc, identf)
            ones64 = persist.tile([1, HD], F32, tag="ones64")
            nc.vector.memset(ones64, 1.0)
            ones11 = persist.tile([1, 1], F32, tag="ones11")
            nc.vector.memset(ones11, 1.0)
            for k in range(QT):
                nc.vector.memset(vsb[k][:, HD:HD + 1], 1.0)

            cb_sb = persist.tile([HD, 1], F32, tag="cb")
            pb_sb = persist.tile([HD, 1], F32, tag="pb")
            nc.sync.dma_start(out=cb_sb, in_=cbias[:, :])
            nc.sync.dma_start(out=pb_sb, in_=pbias[:, :])

            # ---------------- phase 1: projections ----------------
            with contextlib.ExitStack() as ph1:
                wpool = ph1.enter_context(tc.tile_pool(name="wpool", bufs=1))
                xstream = ph1.enter_context(tc.tile_pool(name="xstream", bufs=4))

                # f-interleaved input streaming across three queues
                wq_sb, wkv_sb, xts = [], [], []

                def load_f(f):
                    t = wpool.tile([128, HD], FP16, tag=f"wq{f}", name=f"wq{f}")
                    nc.gpsimd.dma_start(out=t, in_=wq[128 * f:128 * (f + 1), :])
                    wq_sb.append(t)
                    t = wpool.tile([128, 2 * HD], FP16, tag=f"wkv{f}", name=f"wkv{f}")
                    nc.gpsimd.dma_start(out=t, in_=wkv[128 * f:128 * (f + 1), :])
                    wkv_sb.append(t)
                    xt = xstream.tile([128, N], FP16, tag="xt", name=f"xt{f}")
                    nc.scalar.dma_start(out=xt[:, 0:768],
                                        in_=xT[128 * f:128 * (f + 1), 0:768])
                    nc.sync.dma_start(out=xt[:, 768:1536],
                                      in_=xT[128 * f:128 * (f + 1), 768:1536])
                    nc.gpsimd.dma_start(out=xt[:, 1536:2048],
                                        in_=xT[128 * f:128 * (f + 1), 1536:2048])
                    xts.append(xt)

                for f in range(4):
                    load_f(f)

                # phase-1 accumulators borrow the phase-2 tag rings
                # (8 live banks: big x4, avacc x2, pmisc x1, pfin x1 + vtp)
                kv_ps2 = [psum.tile([128, 1024], F32, tag="big", bufs=2,
                                    name=f"kvps{i}") for i in range(2)]
                kv_ps = [kv_ps2[0][:, 0:512], kv_ps2[0][:, 512:1024],
                         kv_ps2[1][:, 0:512], kv_ps2[1][:, 512:1024]]
                q_ps = [psum.tile([HD, 512], F32, tag="avacc", bufs=1,
                                  name="qps0"),
                        psum.tile([HD, 512], F32, tag="tp", bufs=2,
                                  name="qps1"),
                        psum.tile([HD, 512], F32, tag="pmisc", bufs=1,
                                  name="qps2"),
                        psum.tile([HD, 512], F32, tag="tp", bufs=2,
                                  name="qps3")]
                for f in range(12):
                    xt = xts[f]
                    for i in range(4):
                        nc.tensor.matmul(q_ps[i], wq_sb[f],
                                         xt[:, 512 * i:512 * (i + 1)],
                                         start=(f == 0), stop=(f == 11))
                    for i in range(4):
                        nc.tensor.matmul(kv_ps[i], wkv_sb[f],
                                         xt[:, 512 * i:512 * (i + 1)],
                                         start=(f == 0), stop=(f == 11))
                    if f + 4 < 12:
                        load_f(f + 4)
                    elif f == 8:
                        nc.gpsimd.dma_start(out=PT, in_=ptp[:, :])
                ph1_acts = {}
                def q_act(i):
                    nc.scalar.activation(qcT[:, 512 * i:512 * (i + 1)], q_ps[i],
                                         Act.Identity, bias=cb_sb)
                    nc.vector.tensor_scalar_add(qpT[:, 512 * i:512 * (i + 1)],
                                                q_ps[i], pb_sb)
                    nc.vector.tensor_copy(kvT[:, 512 * i:512 * (i + 1)], kv_ps[i])
                ph1_acts["q_act"] = q_act

                def vtps(k0, k1):
                    for k in range(k0, k1):
                        tp = psum.tile([128, HD], FP16, tag="tp", bufs=2,
                                       name=f"vtp{k}")
                        nc.tensor.transpose(tp, kvT[HD:128, 128 * k:128 * (k + 1)],
                                            ident[HD:128, HD:128])
                        nc.vector.tensor_copy(vsb[k][:, 0:HD], tp)
                ph1_acts["vtps"] = vtps

            # out-projection weights (needed from fin chunks)
            wout_sb = []
            for r in range(4):
                t = persist.tile([128, 192], BF16, tag=f"wo{r}", name=f"wo{r}")
                nc.gpsimd.dma_start(out=t, in_=wout[128 * r:128 * (r + 1), :])
                wout_sb.append(t)
            bout_sb = persist.tile([128, 192], F32, tag="bo")
            bout_bc = bass.AP(tensor=bout.ap().tensor, offset=0,
                              ap=[[0, 128], [1, 192]])
            nc.gpsimd.dma_start(out=bout_sb, in_=bout_bc)

            # ---------------- phase 2: scores + softmax + AV + out ---------
            at34 = {}
            otp = {}

            def group_slot(a):
                return a // 4, a % 4

            def produce_mm(a):
                """T matmuls -> tsb fp16 (PE + DVE only)."""
                w0 = 1920 - 128 * a
                tsb = work.tile([128, TSTRIDE], FP16, tag="tsb", bufs=5,
                                name=f"tsb{a}")
                for ci, (off, w) in enumerate(CHUNKS):
                    tp = psum.tile([128, 512], F32, tag="tp", bufs=2,
                                   name=f"tp{a}_{off}")
                    nc.tensor.matmul(tp[:, :w], qpT[:, 128 * a:128 * (a + 1)],
                                     PT[:, w0 + off:w0 + off + w],
                                     start=True, stop=True)
                    nc.vector.tensor_copy(tsb[:, off:off + w], tp[:, :w])
                return tsb

            def t_write(a, tsb, eng=None):
                (eng or nc.gpsimd).dma_start(out=t_dram[a][:, 0:WIN],
                                             in_=tsb[:, 0:WIN])

            def shear_read(a, eng=None):
                shear = work.tile([128, N], FP16, tag="shear", bufs=6,
                                  name=f"shear{a}")
                src = bass.AP(tensor=t_dram[a].ap().tensor, offset=127,
                              ap=[[TSTRIDE - 1, 128], [1, N]])
                (eng or nc.gpsimd).dma_start(out=shear, in_=src)
                return shear

            def consume_t(a, shear):
                """scores = qk (PSUM) + shear (identity-matmul), exp -> an."""
                an = work.tile([128, N], BF16, tag="an", bufs=6, name=f"an{a}")
                sps = [psum.tile([128, 1024], F32, tag="big", bufs=2,
                                 name=f"sp{a}_{h}") for h in range(2)]
                for j in range(4):
                    nc.tensor.matmul(sps[j // 2][:, 512 * (j % 2):512 * (j % 2 + 1)],
                                     qcT[:, 128 * a:128 * (a + 1)],
                                     kvT[0:HD, 512 * j:512 * (j + 1)],
                                     start=True, stop=False)
                for j in range(4):
                    nc.tensor.matmul(sps[j // 2][:, 512 * (j % 2):512 * (j % 2 + 1)],
                                     ident, shear[:, 512 * j:512 * (j + 1)],
                                     start=False, stop=True)
                for h in range(2):
                    nc.scalar.activation(an[:, 1024 * h:1024 * (h + 1)], sps[h],
                                         Act.Exp)
                return an

            def transpose_an(a, an):
                g, jloc = group_slot(a)
                if g not in at34:
                    at34[g] = work.tile([128, QT, 512], BF16, tag="at34", bufs=4,
                                        name=f"at34_{g}")
                nc.sync.dma_start_transpose(
                    at34[g][:, :, 128 * jloc:128 * (jloc + 1)], an)

            ogns = {}

            def av_norm(g):
                """AV for group g + normalize (no collective here)."""
                g0, g1 = GROUPS[g]
                m = g1 - g0
                w = 128 * m
                o = psum.tile([HD + 1, 512], F32, tag="avacc", bufs=1,
                              name=f"otp{g}")
                otp[g] = o
                for k in range(QT):
                    nc.tensor.matmul(o[:, :w], vsb[k], at34[g][:, k, :w],
                                     start=(k == 0), stop=(k == QT - 1))
                # partition-parallel reciprocal of the R row
                rs = work.tile([1, 512], F32, tag="rs", bufs=2, name=f"rs{g}")
                nc.vector.tensor_copy(rs[:, :w], o[HD:HD + 1, :w])
                rcol = psum.tile([128, 4], F32, tag="pmisc", bufs=1,
                                 name=f"rcol{g}")
                for j in range(m):
                    nc.tensor.matmul(rcol[:, j:j + 1],
                                     rs[0:1, 128 * j:128 * (j + 1)], ones11,
                                     start=True, stop=True)
                rci = work.tile([128, 4], F32, tag="rci", bufs=2, name=f"rci{g}")
                nc.vector.reciprocal(rci[:, :m], rcol[:, :m])
                rrow = psum.tile([1, 512], F32, tag="pmisc", bufs=1,
                                 name=f"rrow{g}")
                for j in range(m):
                    nc.tensor.matmul(rrow[0:1, 128 * j:128 * (j + 1)],
                                     rci[:, j:j + 1], identf,
                                     start=True, stop=True)
                rsi = work.tile([1, 512], F32, tag="rsi", bufs=2, name=f"rsi{g}")
                nc.vector.tensor_copy(rsi[:, :w], rrow[0:1, :w])
                bc = psum.tile([HD, 512], F32, tag="pmisc", bufs=1, name=f"bc{g}")
                nc.tensor.matmul(bc[:, :w], ones64, rsi[:, :w],
                                 start=True, stop=True)
                bcs = work.tile([HD, 512], F32, tag="bcs", bufs=2, name=f"bcs{g}")
                nc.vector.tensor_copy(bcs[:, :w], bc[:, :w])
                ogn = work.tile([HD, 512], BF16, tag="ogn", bufs=4, name=f"ogn{g}")
                nc.vector.tensor_mul(ogn[:, :w], o[0:HD, :w], bcs[:, :w])
                ogns[g] = ogn

            def group_ag(g):
                nc.gpsimd.dma_start(out=oag_in[g][:, :], in_=ogns[g])
                nc.gpsimd.collective_compute(
                    "AllGather", mybir.AluOpType.bypass, replica_groups=rgroups,
                    ins=[oag_in[g].ap().opt()], outs=[oag_out[g].ap().opt()])

            def fin_group(g):
                """out rows for group g = gathered-O^T @ Wout + b."""
                ofull = []
                for r in range(4):
                    t = work.tile([128, 512], BF16, tag="of", bufs=8,
                                  name=f"of{g}{r}")
                    nc.gpsimd.dma_start(out=t,
                                        in_=oag_out[g][128 * r:128 * (r + 1), :])
                    ofull.append(t)
                for mm in range(4):
                    fp = psum.tile([128, 192], F32, tag="tp", bufs=2,
                                   name=f"fp{g}_{mm}")
                    for r in range(4):
                        nc.tensor.matmul(fp, ofull[r][:, 128 * mm:128 * (mm + 1)],
                                         wout_sb[r], start=(r == 0), stop=(r == 3))
                    ob = work.tile([128, 192], FP16, tag="ob", bufs=4,
                                   name=f"ob{g}_{mm}")
                    nc.vector.tensor_add(ob, fp, bout_sb)
                    row = 4 * g + mm
                    nc.gpsimd.dma_start(out=out_ext[128 * row:128 * (row + 1), :],
                                        in_=ob)

            # ---------------- schedule ----------------
            # software pipeline over tiles 0..15: consume(i) | transpose(i-1)
            # | T-mms(i+4) | T-write(i+3) | shear-read(i+2).  NO collective
            # overlaps the scores phase (the xbar transposes are serialized
            # against collectives by the framework), then AV groups +
            # normalize + two super-chunk AllGathers + final projections.
            q_act = ph1_acts["q_act"]
            tsbs, shears, ans = {}, {}, {}
            q_act(0)
            q_act(1)
            tsbs[0] = produce_mm(0)
            q_act(3)
            tsbs[1] = produce_mm(1)
            q_act(2)
            tsbs[2] = produce_mm(2)
            tsbs[3] = produce_mm(3)
            t_write(0, tsbs.pop(0), eng=nc.scalar)
            t_write(1, tsbs.pop(1), eng=nc.sync)
            t_write(2, tsbs.pop(2), eng=nc.scalar)
            shears[0] = shear_read(0, eng=nc.sync)
            shears[1] = shear_read(1, eng=nc.scalar)
            vtps = ph1_acts["vtps"]
            for a in range(QT):
                ans[a] = consume_t(a, shears.pop(a))
                if a < 4:
                    vtps(4 * a, 4 * (a + 1))
                if a >= 1:
                    transpose_an(a - 1, ans.pop(a - 1))
                if a + 2 < QT:
                    shears[a + 2] = shear_read(a + 2)
                if a + 4 < QT:
                    tsbs[a + 4] = produce_mm(a + 4)
                if a + 3 < QT:
                    t_write(a + 3, tsbs.pop(a + 3))
            transpose_an(QT - 1, ans.pop(QT - 1))
            av_norm(0)
            group_ag(0)
            av_norm(1)
            group_ag(1)
            av_norm(2)
            group_ag(2)
            av_norm(3)
            group_ag(3)
            fin_group(0)
            fin_group(1)
            fin_group(2)
            fin_group(3)

    # wait-split post-processing hook
    orig = nc.to_json_bytes
    nc.to_json_bytes = lambda: _split_waits(orig())
    _GRAPH_CACHE["nc"] = nc
    return nc


# ----------------------------------------------------------------------------
def _prep_inputs(x, Wq, Wk, Wv, content_bias, pos_bias, Wp_w, Wp_b, Wout_w, Wout_b):
    x = np.ascontiguousarray(np.asarray(x, dtype=np.float32))
    Wq = np.asarray(Wq, np.float32); Wk = np.asarray(Wk, np.float32)
    Wv = np.asarray(Wv, np.float32)
    content_bias = np.asarray(content_bias, np.float32)
    pos_bias = np.asarray(pos_bias, np.float32)
    Wp_w = np.asarray(Wp_w, np.float32); Wp_b = np.asarray(Wp_b, np.float32)
    Wout_w = np.asarray(Wout_w, np.float32); Wout_b = np.asarray(Wout_b, np.float32)

    scale = HD ** -0.5
    xT = np.ascontiguousarray(x[0].T)                    # (1536, 2048)
    emb = _pos_embed()                                   # (4095, 1536)
    # weight-only pos projection, summed over heads (faithful to the
    # reference einsum which contracts the head axis of pos)
    wp_sum = Wp_w.reshape(DM, H, HD).sum(axis=1)         # (1536, 64)
    wp_b_sum = Wp_b.reshape(H, HD).sum(axis=0)           # (64,)
    posproj = emb @ wp_sum + wp_b_sum                    # (4095, 64) fp32
    ptp = np.zeros((HD, 4096), np.float16)
    ptp[:, :2 * N - 1] = posproj.T.astype(np.float16)
    xT16 = np.ascontiguousarray(xT).astype(np.float16)

    in_maps = []
    for c in range(NCORES):
        sl = slice(HD * c, HD * (c + 1))
        in_maps.append({
            "xT": xT16,
            "wq": np.ascontiguousarray(Wq[:, sl] * scale).astype(np.float16),
            "wkv": np.ascontiguousarray(
                np.concatenate([Wk[:, sl], Wv[:, sl]], axis=1)).astype(np.float16),
            "ptp": ptp,
            "cbias": np.ascontiguousarray(content_bias[c, 0, :, None]),
            "pbias": np.ascontiguousarray(pos_bias[c, 0, :, None]),
            "wout": np.ascontiguousarray(
                Wout_w[:, 192 * c:192 * (c + 1)]).astype(ml_dtypes.bfloat16),
            "bout": np.ascontiguousarray(Wout_b[None, 192 * c:192 * (c + 1)]),
        })
    return in_maps


def kernel(x, Wq, Wk, Wv, content_bias, pos_bias, Wp_w, Wp_b, Wout_w, Wout_b):
    global _LAST_RESULT
    in_maps = _prep_inputs(x, Wq, Wk, Wv, content_bias, pos_bias,
                           Wp_w, Wp_b, Wout_w, Wout_b)
    nc = _build_graph()
    trace = bool(os.environ.get("KERNEL_TRACE"))
    res = run_bass_kernel_spmd(nc, in_maps, core_ids=list(range(NCORES)),
                               trace=trace, trace_cores=[0] if trace else None)
    _LAST_RESULT = res
    out = np.concatenate([np.asarray(res.results[c]["out"])
                          for c in range(NCORES)], axis=1)
    return out[None].astype(np.float32)
